# revision 13
# baseline (speedup 1.0000x reference)
"""Bilinear crop-resize (ImageInterpolator) Trainium2 Bass kernel.

Full inputs: image (64, 512, 512, 8) f32, section (64, 3) f32.
Output: (64, 64, 64, 8) f32 bilinear crop per batch on a 64x64 grid.

Sharding: data parallel over batch, 8 batches per core on 8 cores.

Per-core device pipeline, per batch b:
  1. Compute gather row indices on device from `section` (128 rows:
     64 lo rows then 64 hi rows, globally offset by b*512).
  2. indirect DMA gather -> G (128 slots, 4096 f32)  [rows are 16KB contig]
  3. Row blend as matmul:  Rb(64i, (w,c)) = BlendM.T @ G   (fp32r, N=512)
     BlendM[slot, i] = (slot==i)*(1-w0[i]) + (slot==64+i)*w0[i]
  4. Copy PSUM->SBUF with cast to bf16.
  5. TensorE transposes per (k-chunk, c): Rb(64, 128 stride 8) -> G2(128w, c*64+i)
  6. Column interp as matmul vs tent weights:
     Mcol[w', j] = relu(1 - |i1[j] - w|),  out accum over 4 w-chunks.
  7. Copy PSUM -> out_sb (64, j*8+c) and store contiguous 128KB.
"""

import sys

sys.path.insert(0, "/opt/trn_rl_repo")

import numpy as np

import concourse.bacc as bacc
import concourse.bass as bass
import concourse.mybir as mybir
import concourse.tile as tile
from concourse.bass import IndirectOffsetOnAxis
from concourse.masks import make_identity

F32 = mybir.dt.float32
F32R = mybir.dt.float32r
BF16 = mybir.dt.bfloat16
I32 = mybir.dt.int32
ALU = mybir.AluOpType
ACT = mybir.ActivationFunctionType

B_PER_CORE = 8
H = 512
W = 512
C = 8
G0 = 64
G1 = 64
WC = W * C  # 4096 row elements
N_CORES = 8


def build_program():
    nc = bacc.Bacc("TRN2", target_bir_lowering=False, debug=False)

    image = nc.dram_tensor("image", (B_PER_CORE * H, WC), F32, kind="ExternalInput")
    section = nc.dram_tensor("section", (B_PER_CORE, 3), F32, kind="ExternalInput")
    out_d = nc.dram_tensor("out", (B_PER_CORE * G0, G1 * C), F32, kind="ExternalOutput")

    with tile.TileContext(nc) as tc:
        with (
            tc.tile_pool(name="const", bufs=1) as cpool,
            tc.tile_pool(name="gpool", bufs=2) as gpool,
            tc.tile_pool(name="rbpool", bufs=2) as rbpool,
            tc.tile_pool(name="g2pool", bufs=2) as g2pool,
            tc.tile_pool(name="mpool", bufs=2) as mpool,
            tc.tile_pool(name="opool", bufs=2) as opool,
            tc.tile_pool(name="ps1", bufs=3, space="PSUM") as ps1pool,
            tc.tile_pool(name="psT", bufs=2, space="PSUM") as psTpool,
            tc.tile_pool(name="ps2", bufs=2, space="PSUM") as ps2pool,
            tc.tile_pool(name="psb", bufs=1, space="PSUM") as psbpool,
        ):
            # ---------------- one-time setup ----------------
            ident = cpool.tile([128, 128], BF16)
            make_identity(nc, ident[:])

            ones_row = cpool.tile([1, 128], F32)
            nc.gpsimd.memset(ones_row[:], 1.0)

            # section: flat (1,24) for broadcast + transposed rows (1,8)
            sec_t = section[:].rearrange("b k -> k b")  # (3, 8) DRAM view
            s1f = cpool.tile([1, B_PER_CORE], F32)
            nc.sync.dma_start(out=s1f[:], in_=sec_t[1:2, :])
            scf = cpool.tile([1, B_PER_CORE], F32)
            nc.sync.dma_start(out=scf[:], in_=sec_t[2:3, :])
            sec_flat = cpool.tile([1, 3 * B_PER_CORE], F32)
            nc.sync.dma_start(out=sec_flat[:], in_=section[:].rearrange("b k -> (b k)")[None, :])
            # broadcast section to all 128 partitions via K=1 ones-matmul
            sbc_ps = psbpool.tile([128, B_PER_CORE * G1], F32, tag="bcast")
            nc.tensor.matmul(sbc_ps[:, : 3 * B_PER_CORE], lhsT=ones_row[:], rhs=sec_flat[:], start=True, stop=True)
            sbc = cpool.tile([128, 3 * B_PER_CORE], F32)
            nc.vector.tensor_copy(out=sbc[:], in_=sbc_ps[:, : 3 * B_PER_CORE])
            sbc3 = sbc[:].rearrange("p (b k) -> p b k", k=3)

            # iota constants
            iota_p_i = cpool.tile([128, 1], I32)
            nc.gpsimd.iota(iota_p_i[:], pattern=[[0, 1]], channel_multiplier=1)
            iota_p = cpool.tile([128, 1], F32)
            nc.vector.tensor_copy(out=iota_p[:], in_=iota_p_i[:])
            # ge = 1.0 for hi-row partitions (p >= 64)
            ge_f = cpool.tile([128, 1], F32)
            nc.vector.tensor_scalar(out=ge_f[:], in0=iota_p[:], scalar1=63.5, scalar2=None, op0=ALU.is_gt)
            # t_p = (p mod 64)/63  on partitions
            t_p = cpool.tile([128, 1], F32)
            nc.vector.scalar_tensor_tensor(out=t_p[:], in0=ge_f[:], scalar=-64.0, in1=iota_p[:], op0=ALU.mult, op1=ALU.add)
            nc.vector.tensor_scalar(out=t_p[:], in0=t_p[:], scalar1=1.0 / 63.0, scalar2=None, op0=ALU.mult)

            # t1 for all batches on one row: (1, 8*64) = 8 copies of 0..63
            t1_i = cpool.tile([1, B_PER_CORE * G1], I32)
            nc.gpsimd.iota(t1_i[:], pattern=[[0, B_PER_CORE], [1, G1]], channel_multiplier=0)
            t1big = cpool.tile([1, B_PER_CORE * G1], F32)
            nc.vector.tensor_copy(out=t1big[:], in_=t1_i[:])
            nc.vector.tensor_scalar(out=t1big[:], in0=t1big[:], scalar1=1.0 / 63.0, scalar2=None, op0=ALU.mult)

            # w-grid base per chunk: wgrid[p, k] = p + 128k
            wgrid_i = cpool.tile([128, 4], I32)
            nc.gpsimd.iota(wgrid_i[:], pattern=[[128, 4]], channel_multiplier=1)
            wgrid = cpool.tile([128, 4], F32)
            nc.vector.tensor_copy(out=wgrid[:], in_=wgrid_i[:])

            # ID_lo / D = ID_hi - ID_lo  (slot-selection diagonals)
            iota_pi = cpool.tile([128, G0], I32)  # value = f - p
            nc.gpsimd.iota(iota_pi[:], pattern=[[1, G0]], channel_multiplier=-1)
            id_lo = cpool.tile([128, G0], F32)
            nc.vector.tensor_scalar(out=id_lo[:], in0=iota_pi[:], scalar1=0, scalar2=None, op0=ALU.is_equal)
            id_hi = cpool.tile([128, G0], F32)
            nc.vector.tensor_scalar(out=id_hi[:], in0=iota_pi[:], scalar1=-64, scalar2=None, op0=ALU.is_equal)
            d_st = cpool.tile([128, G0], F32)
            nc.vector.tensor_tensor(out=d_st[:], in0=id_hi[:], in1=id_lo[:], op=ALU.subtract)

            # badd[p, b] = 512*b replicated on all partitions
            badd_i = cpool.tile([128, B_PER_CORE], I32)
            nc.gpsimd.iota(badd_i[:], pattern=[[H, B_PER_CORE]], channel_multiplier=0)
            badd = cpool.tile([128, B_PER_CORE], F32)
            nc.vector.tensor_copy(out=badd[:], in_=badd_i[:])

            # ---------------- all-batch index computation ----------------
            # q0a[p, b] = start0[b] + t_p[p] * scale[b]
            q0a = cpool.tile([128, B_PER_CORE], F32)
            nc.vector.scalar_tensor_tensor(
                out=q0a[:], in0=sbc3[:, :, 2], scalar=t_p[:, 0:1], in1=sbc3[:, :, 0],
                op0=ALU.mult, op1=ALU.add,
            )
            # i0a = clip(q0a * 511, 0, 511)
            i0a = cpool.tile([128, B_PER_CORE], F32)
            nc.vector.tensor_scalar(out=i0a[:], in0=q0a[:], scalar1=float(H - 1), scalar2=0.0, op0=ALU.mult, op1=ALU.max)
            nc.vector.tensor_scalar(out=i0a[:], in0=i0a[:], scalar1=float(H - 1), scalar2=None, op0=ALU.min)
            # floor(i0a) robust to int-cast rounding mode: cast, cast back,
            # subtract 1 where the round went up. moda = frac = w0.
            fl_i = cpool.tile([128, B_PER_CORE], I32)
            nc.vector.tensor_copy(out=fl_i[:], in_=i0a[:])
            fl_f = cpool.tile([128, B_PER_CORE], F32)
            nc.vector.tensor_copy(out=fl_f[:], in_=fl_i[:])
            gta = cpool.tile([128, B_PER_CORE], F32)
            nc.vector.tensor_tensor(out=gta[:], in0=fl_f[:], in1=i0a[:], op=ALU.is_gt)
            rowa = cpool.tile([128, B_PER_CORE], F32)
            nc.vector.tensor_tensor(out=rowa[:], in0=fl_f[:], in1=gta[:], op=ALU.subtract)
            moda = cpool.tile([128, B_PER_CORE], F32)
            nc.vector.tensor_tensor(out=moda[:], in0=i0a[:], in1=rowa[:], op=ALU.subtract)
            # row = min(floor + ge, 511) + 512*b
            nc.vector.tensor_scalar(out=rowa[:], in0=rowa[:], scalar1=ge_f[:, 0:1], scalar2=float(H - 1), op0=ALU.add, op1=ALU.min)
            nc.vector.tensor_tensor(out=rowa[:], in0=rowa[:], in1=badd[:], op=ALU.add)
            idx_all = cpool.tile([128, B_PER_CORE], I32)
            nc.vector.tensor_copy(out=idx_all[:], in_=rowa[:])

            # i1 for all batches in one row then broadcast to 128 partitions:
            # i1flat[0, b*64+j] = clip((start1[b] + t1[j]*scale[b]) * 511)
            i1flat = cpool.tile([1, B_PER_CORE * G1], F32)
            t1v = t1big[:].rearrange("p (b j) -> p b j", b=B_PER_CORE)
            i1v = i1flat[:].rearrange("p (b j) -> p b j", b=B_PER_CORE)
            scv = scf[:][:, :, None].to_broadcast([1, B_PER_CORE, G1])
            s1v = s1f[:][:, :, None].to_broadcast([1, B_PER_CORE, G1])
            nc.vector.tensor_tensor(out=i1v, in0=t1v, in1=scv, op=ALU.mult)
            nc.vector.tensor_tensor(out=i1v, in0=i1v, in1=s1v, op=ALU.add)
            nc.vector.tensor_scalar(out=i1flat[:], in0=i1flat[:], scalar1=float(W - 1), scalar2=0.0, op0=ALU.mult, op1=ALU.max)
            nc.vector.tensor_scalar(out=i1flat[:], in0=i1flat[:], scalar1=float(W - 1), scalar2=None, op0=ALU.min)
            i1bc_ps = psbpool.tile([128, B_PER_CORE * G1], F32, tag="bcast")
            nc.tensor.matmul(i1bc_ps[:], lhsT=ones_row[:], rhs=i1flat[:], start=True, stop=True)
            i1sb = cpool.tile([128, B_PER_CORE * G1], F32)
            nc.vector.tensor_copy(out=i1sb[:], in_=i1bc_ps[:])

            # ---------------- per-batch pipeline ----------------
            for b in range(B_PER_CORE):
                # 1) gather 128 rows
                g = gpool.tile([128, WC], BF16, tag="g")
                nc.gpsimd.indirect_dma_start(
                    out=g[:], out_offset=None,
                    in_=image[:],
                    in_offset=IndirectOffsetOnAxis(ap=idx_all[:, b : b + 1], axis=0),
                )

                # 2) BlendM = ID_lo + D * w0  (w0 per-partition from moda)
                blendm = mpool.tile([128, G0], BF16, tag="blendm")
                nc.vector.scalar_tensor_tensor(
                    out=blendm[:], in0=d_st[:], scalar=moda[:, b : b + 1],
                    in1=id_lo[:], op0=ALU.mult, op1=ALU.add,
                )

                # 3) stage1 row-blend matmuls (fp32r) + cast copies
                rb = rbpool.tile([G0, WC], BF16, tag="rb")
                for t in range(8):
                    ps1 = ps1pool.tile([G0, 512], F32, tag="ps1")
                    nc.tensor.matmul(
                        ps1[:],
                        lhsT=blendm[:],
                        rhs=g[:, 512 * t : 512 * (t + 1)],
                        start=True, stop=True,
                    )
                    if t % 2 == 0:
                        nc.scalar.copy(out=rb[:, 512 * t : 512 * (t + 1)], in_=ps1[:])
                    else:
                        nc.vector.tensor_copy(out=rb[:, 512 * t : 512 * (t + 1)], in_=ps1[:])

                # 4) Mcol tent weights (128, 4*64) bf16
                dsub = mpool.tile([128, 4 * G1], F32, tag="dsub")
                for k in range(4):
                    nc.vector.tensor_scalar(
                        out=dsub[:, G1 * k : G1 * (k + 1)],
                        in0=i1sb[:, G1 * b : G1 * (b + 1)],
                        scalar1=wgrid[:, k : k + 1], scalar2=None, op0=ALU.subtract,
                    )
                absd = mpool.tile([128, 4 * G1], F32, tag="absd")
                nc.scalar.activation(out=absd[:], in_=dsub[:], func=ACT.Abs)
                mcol = mpool.tile([128, 4 * G1], BF16, tag="mcol")
                nc.scalar.activation(out=mcol[:], in_=absd[:], func=ACT.Relu, bias=1.0, scale=-1.0)

                # 5) transposes: Rb (64, 128 stride 8) -> G2 (128 w', c*64+i) per chunk
                rb4 = rb[:].rearrange("p (k w c) -> p k w c", k=4, w=128, c=C)
                g2 = g2pool.tile([128, 4 * 512], BF16, tag="g2")
                for k in range(4):
                    psT = psTpool.tile([128, 512], BF16, tag="psT")
                    for c in range(C):
                        nc.tensor.transpose(
                            out=psT[:, G0 * c : G0 * (c + 1)],
                            in_=rb4[:, k, :, c],
                            identity=ident[0:G0, 0:G0],
                        )
                    if k % 2 == 0:
                        nc.scalar.copy(out=g2[:, 512 * k : 512 * (k + 1)], in_=psT[:])
                    else:
                        nc.vector.tensor_copy(out=g2[:, 512 * k : 512 * (k + 1)], in_=psT[:])

                # 6) column-interp matmuls, accumulate over 4 w-chunks
                ps2 = ps2pool.tile([128, 4 * G1], F32, tag="ps2")
                for cp in range(4):
                    for k in range(4):
                        nc.tensor.matmul(
                            ps2[:, G1 * cp : G1 * (cp + 1)],
                            lhsT=g2[:, 512 * k + 128 * cp : 512 * k + 128 * (cp + 1)],
                            rhs=mcol[:, G1 * k : G1 * (k + 1)],
                            start=(k == 0), stop=(k == 3),
                        )

                # 7) interleave copies into (64, j*8+c) + store
                out_sb = opool.tile([G0, G1 * C], F32, tag="outsb")
                osv = out_sb[:].rearrange("p (j c) -> p j c", c=C)
                for cp in range(4):
                    for c2 in range(2):
                        cc = 2 * cp + c2
                        src = ps2[64 * c2 : 64 * (c2 + 1), G1 * cp : G1 * (cp + 1)]
                        if cc % 2 == 0:
                            nc.scalar.copy(out=osv[:, :, cc], in_=src)
                        else:
                            nc.vector.tensor_copy(out=osv[:, :, cc], in_=src)
                nc.sync.dma_start(out=out_d[G0 * b : G0 * (b + 1), :], in_=out_sb[:])

    nc.compile()
    return nc


_NC_CACHE = None


def _get_nc():
    global _NC_CACHE
    if _NC_CACHE is None:
        _NC_CACHE = build_program()
    return _NC_CACHE


def kernel(image: np.ndarray, section: np.ndarray) -> np.ndarray:
    from concourse.bass_utils import run_bass_kernel_spmd

    image = np.ascontiguousarray(image, dtype=np.float32)
    section = np.ascontiguousarray(section, dtype=np.float32)
    B = image.shape[0]
    assert B == B_PER_CORE * N_CORES

    nc = _get_nc()
    in_maps = []
    for c in range(N_CORES):
        sl = slice(c * B_PER_CORE, (c + 1) * B_PER_CORE)
        in_maps.append(
            {
                "image": image[sl].reshape(B_PER_CORE * H, WC),
                "section": np.ascontiguousarray(section[sl]),
            }
        )
    res = run_bass_kernel_spmd(nc, in_maps, core_ids=list(range(N_CORES)))
    outs = [r["out"].reshape(B_PER_CORE, G0, G1, C) for r in res.results]
    return np.concatenate(outs, axis=0)
